# revision 5
# baseline (speedup 1.0000x reference)
"""Trainium2 Bass kernel for autoregressive GRU sampling (stacked-halves v2).

Problem: B=16384 samples, 1024 sequential sites, hidden=64, PyTorch GRU-cell
math with gates [r,z,n], Bernoulli sampling via pre-drawn uniforms.

Data-parallel over 8 cores (2048 samples/core). Per core, 2 independent
streams of 1024 samples pipeline against each other; within a stream the two
512-sample halves A/B are STACKED on partitions, so every elementwise tensor
is [128, 512] (rows 0-63 = half A, rows 64-127 = half B). DVE/ACT/GPSIMD are
free-dim streaming engines, so this halves their per-op time twice over vs
the wide [64, 2048] layout, at the cost of K=1 matmuls for the bit/uhat
contributions (PE has headroom).

Cell math per site, with c := sigma(-a_z) so h' = h + c*(n - h):
  P_c = -(Wz h + bz + bit wz)      K=64 mm (bias via ones... folded into mm? no:
                                    bias folded into K=1 path) -- see packing
  P_r = Wr h + br + bit wr
  P_n = Wn h                        (gh, pre r-gating)
  P_gx = bit w_ihn                  (gx)
  C = sigmoid(P_c); R = sigmoid(P_r)          [ACT, bias port adds gate bias]
  RG = (P_n + bhn) * R                        [DVE stt]
  NP = RG + P_gx                              [DVE tt]
  NT = tanh(NP + bin)                         [ACT bias port]
  E = NT - H                                  [GPSIMD]
  U = C * E                                   [GPSIMD]
  H' = H + U                                  [DVE]
  ph = wh . h' - uhat_t   (psum [33,512], A at row 0, B at row 32)
  BIT' = (ph > 0)                             [DVE tensor_scalar is_gt]
Host precomputes uhat = logit(u) - head_b in float64.
"""

import numpy as np
from contextlib import ExitStack

HIDDEN = 64
N_SITES = 1024
BATCH = 16384
N_CORES = 8
B_LOCAL = BATCH // N_CORES  # 2048
NSTREAM = 2
NH = 512  # half-size: samples per stacked half

_BUILD_CACHE = {}


def _build(n_sites: int, compile: bool = True):
    import concourse.bass as bass
    import concourse.bacc as bacc
    import concourse.tile as tile
    from concourse import mybir

    f32 = mybir.dt.float32
    AF = mybir.ActivationFunctionType
    OP = mybir.AluOpType

    nc = bacc.Bacc()
    # [stream, site, half, NH]
    uhat_d = nc.dram_tensor("uhat", [NSTREAM, n_sites, 2, NH], f32, kind="ExternalInput")
    wc_d = nc.dram_tensor("wc", [128, 64], f32, kind="ExternalInput")
    wr_d = nc.dram_tensor("wr", [128, 64], f32, kind="ExternalInput")
    wn_d = nc.dram_tensor("wn", [128, 64], f32, kind="ExternalInput")
    whd_d = nc.dram_tensor("whd", [128, 1], f32, kind="ExternalInput")
    wbc_d = nc.dram_tensor("wbc", [33, 64], f32, kind="ExternalInput")
    wbr_d = nc.dram_tensor("wbr", [33, 64], f32, kind="ExternalInput")
    wbn_d = nc.dram_tensor("wbn", [33, 64], f32, kind="ExternalInput")
    wu_d = nc.dram_tensor("wu", [33, 1], f32, kind="ExternalInput")
    bc_d = nc.dram_tensor("bc", [128, 1], f32, kind="ExternalInput")
    br_d = nc.dram_tensor("br", [128, 1], f32, kind="ExternalInput")
    bhn_d = nc.dram_tensor("bhn", [128, 1], f32, kind="ExternalInput")
    bin_d = nc.dram_tensor("bin", [128, 1], f32, kind="ExternalInput")
    # bits out: [site, stream*1024 + half*512 + i] -- host reorders columns
    bits_d = nc.dram_tensor("bits", [n_sites, B_LOCAL], f32, kind="ExternalOutput")

    with ExitStack() as ctx:
        tc = ctx.enter_context(tile.TileContext(nc))
        const = ctx.enter_context(tc.tile_pool(name="const", bufs=1))
        work = ctx.enter_context(tc.tile_pool(name="work", bufs=2))
        # one psum bank per tag-slot; 4 slots per stream
        psum = [
            ctx.enter_context(tc.tile_pool(name=f"ps{s}", bufs=1, space="PSUM"))
            for s in range(NSTREAM)
        ]

        # Weights/biases bounce through DVE copies so consumers' sem waits
        # collapse onto the DVE counter (keeps matmul <=2 sync waits).
        w_tiles = {}
        for nm, dram, shp in (
            ("wc", wc_d, [128, 64]), ("wr", wr_d, [128, 64]), ("wn", wn_d, [128, 64]),
            ("whd", whd_d, [128, 1]), ("wbc", wbc_d, [33, 64]),
            ("wbr", wbr_d, [33, 64]), ("wbn", wbn_d, [33, 64]), ("wu", wu_d, [33, 1]),
            ("bc", bc_d, [128, 1]), ("br", br_d, [128, 1]),
            ("bhn", bhn_d, [128, 1]), ("bin", bin_d, [128, 1]),
        ):
            raw = const.tile(shp, f32, name=f"{nm}_raw")
            nc.sync.dma_start(raw[:], dram[:])
            t = const.tile(shp, f32, name=nm)
            nc.vector.tensor_copy(t[:], raw[:])
            w_tiles[nm] = t
        wc, wr, wn = w_tiles["wc"], w_tiles["wr"], w_tiles["wn"]
        whd, wbc, wbr = w_tiles["whd"], w_tiles["wbc"], w_tiles["wbr"]
        wbn, wu = w_tiles["wbn"], w_tiles["wu"]
        bc, br, bhn, bin_ = w_tiles["bc"], w_tiles["br"], w_tiles["bhn"], w_tiles["bin"]

        # Per-stream state: ping-pong pairs
        H = [[const.tile([128, NH], f32, name=f"H{s}_{p}") for p in range(2)]
             for s in range(NSTREAM)]
        BIT = [[const.tile([33, NH], f32, name=f"BIT{s}_{p}") for p in range(2)]
               for s in range(NSTREAM)]
        UH = [[const.tile([33, NH], f32, name=f"UH{s}_{p}") for p in range(2)]
              for s in range(NSTREAM)]
        for s in range(NSTREAM):
            for p in range(2):
                nc.vector.memzero(H[s][p][:])
                nc.vector.memzero(BIT[s][p][0:1, :])
                nc.vector.memzero(BIT[s][p][32:33, :])
            # pre-stage uhat for sites 0 and 1
            for p in range(min(2, n_sites)):
                nc.sync.dma_start(UH[s][p][0:1, :], uhat_d[s, p, 0:1, :])
                nc.sync.dma_start(UH[s][p][32:33, :], uhat_d[s, p, 1:2, :])

        MM = nc.tensor.matmul
        for t in range(n_sites):
            cur = [H[s][t % 2] for s in range(NSTREAM)]
            nxt = [H[s][(t + 1) % 2] for s in range(NSTREAM)]
            bcur = [BIT[s][t % 2] for s in range(NSTREAM)]
            bnxt = [BIT[s][(t + 1) % 2] for s in range(NSTREAM)]
            ucur = [UH[s][t % 2] for s in range(NSTREAM)]

            pc, pr, pn, gx, ph = [], [], [], [], []
            for s in range(NSTREAM):
                c = cur[s]
                b = bcur[s]
                P_c = psum[s].tile([128, NH], f32, tag="pc", name=f"pc{s}")
                P_r = psum[s].tile([128, NH], f32, tag="pr", name=f"pr{s}")
                P_n = psum[s].tile([128, NH], f32, tag="pn", name=f"pn{s}")
                P_gx = psum[s].tile([128, NH], f32, tag="gx", name=f"gx{s}")
                pc.append(P_c); pr.append(P_r); pn.append(P_n); gx.append(P_gx)

                # K=64 gate matmuls (rhs = h of this site), halves A/B
                MM(P_c[0:64, :], wc[0:64, :], c[0:64, :], start=True, stop=False)
                MM(P_c[64:128, :], wc[64:128, :], c[64:128, :], start=True, stop=False)
                MM(P_r[0:64, :], wr[0:64, :], c[0:64, :], start=True, stop=False)
                MM(P_r[64:128, :], wr[64:128, :], c[64:128, :], start=True, stop=False)
                MM(P_n[0:64, :], wn[0:64, :], c[0:64, :], start=True, stop=True)
                MM(P_n[64:128, :], wn[64:128, :], c[64:128, :], start=True, stop=True)
                # K=1 bit contributions (accumulate into gate psums)
                MM(P_c[0:64, :], wbc[0:1, :], b[0:1, :], start=False, stop=True)
                MM(P_c[64:128, :], wbc[32:33, :], b[32:33, :], start=False, stop=True)
                MM(P_r[0:64, :], wbr[0:1, :], b[0:1, :], start=False, stop=True)
                MM(P_r[64:128, :], wbr[32:33, :], b[32:33, :], start=False, stop=True)
                MM(P_gx[0:64, :], wbn[0:1, :], b[0:1, :], start=True, stop=True)
                MM(P_gx[64:128, :], wbn[32:33, :], b[32:33, :], start=True, stop=True)

            C, R, NT = [], [], []
            for s in range(NSTREAM):
                Ct = work.tile([128, NH], f32, tag=f"C{s}", name=f"C{s}")
                Rt = work.tile([128, NH], f32, tag=f"R{s}", name=f"R{s}")
                nc.scalar.activation(Ct[:], pc[s][:], AF.Sigmoid, bias=bc[:])
                nc.scalar.activation(Rt[:], pr[s][:], AF.Sigmoid, bias=br[:])
                C.append(Ct); R.append(Rt)

            RG, NP = [], []
            for s in range(NSTREAM):
                RGt = work.tile([128, NH], f32, tag=f"RG{s}", name=f"RG{s}")
                nc.vector.scalar_tensor_tensor(
                    RGt[:], pn[s][:], bhn[:], R[s][:], OP.add, OP.mult
                )
                NPt = work.tile([128, NH], f32, tag=f"NP{s}", name=f"NP{s}")
                nc.vector.tensor_tensor(NPt[:], RGt[:], gx[s][:], OP.add)
                RG.append(RGt); NP.append(NPt)

            for s in range(NSTREAM):
                NTt = work.tile([128, NH], f32, tag=f"NT{s}", name=f"NT{s}")
                nc.scalar.activation(NTt[:], NP[s][:], AF.Tanh, bias=bin_[:])
                NT.append(NTt)

            E, U = [], []
            for s in range(NSTREAM):
                Et = work.tile([128, NH], f32, tag=f"E{s}", name=f"E{s}")
                nc.gpsimd.tensor_tensor(Et[:], NT[s][:], cur[s][:], OP.subtract)
                Ut = work.tile([128, NH], f32, tag=f"U{s}", name=f"U{s}")
                nc.gpsimd.tensor_tensor(Ut[:], C[s][:], Et[:], OP.mult)
                E.append(Et); U.append(Ut)

            for s in range(NSTREAM):
                nc.vector.tensor_tensor(nxt[s][:], cur[s][:], U[s][:], OP.add)

            for s in range(NSTREAM):
                P_h = psum[s].tile([33, NH], f32, tag="gx", name=f"ph{s}")
                ph.append(P_h)
                MM(P_h[0:1, :], whd[0:64, :], nxt[s][0:64, :], start=True, stop=False)
                MM(P_h[32:33, :], whd[64:128, :], nxt[s][64:128, :], start=True, stop=False)
                MM(P_h[0:1, :], wu[0:1, :], ucur[s][0:1, :], start=False, stop=True)
                MM(P_h[32:33, :], wu[32:33, :], ucur[s][32:33, :], start=False, stop=True)

            for s in range(NSTREAM):
                nc.vector.tensor_scalar(
                    bnxt[s][0:1, :], ph[s][0:1, :], 0.0, None, OP.is_gt
                )
                nc.vector.tensor_scalar(
                    bnxt[s][32:33, :], ph[s][32:33, :], 0.0, None, OP.is_gt
                )

            for s in range(NSTREAM):
                base = s * 1024
                nc.sync.dma_start(
                    bits_d[t : t + 1, base : base + 512], bnxt[s][0:1, :]
                )
                nc.sync.dma_start(
                    bits_d[t : t + 1, base + 512 : base + 1024], bnxt[s][32:33, :]
                )

            # prefetch uhat for site t+2 -- emitted after this site's uhat
            # matmuls read the tile, so the WAR edge orders correctly
            for s in range(NSTREAM):
                if t + 2 < n_sites:
                    nc.sync.dma_start(UH[s][t % 2][0:1, :], uhat_d[s, t + 2, 0:1, :])
                    nc.sync.dma_start(UH[s][t % 2][32:33, :], uhat_d[s, t + 2, 1:2, :])

    if compile:
        nc.compile()
    return nc


def _pack_inputs(u, w_ih, w_hh, b_ih, b_hh, head_w, head_b):
    H = HIDDEN
    w_ih = np.asarray(w_ih, np.float32)
    w_hh = np.asarray(w_hh, np.float32)
    b_ih = np.asarray(b_ih, np.float32)
    b_hh = np.asarray(b_hh, np.float32)
    head_w = np.asarray(head_w, np.float32)
    head_b = np.asarray(head_b, np.float32)

    rows_r = slice(0, H)
    rows_z = slice(H, 2 * H)
    rows_n = slice(2 * H, 3 * H)

    def dup(x):  # [64, M] -> [128, M] duplicated for halves A/B
        return np.concatenate([x, x], axis=0).astype(np.float32)

    wc = dup(-w_hh[rows_z, :].T)            # [128, 64] (c = sigma(-a_z))
    wr = dup(w_hh[rows_r, :].T)
    wn = dup(w_hh[rows_n, :].T)
    whd = dup(head_w[0, :, None])           # [128, 1]

    def k1(vec_m):  # [M] -> [33, M] rows 0 and 32
        out = np.zeros((33, len(vec_m)), np.float32)
        out[0, :] = vec_m
        out[32, :] = vec_m
        return out

    wbc = k1(-w_ih[rows_z, 0])
    wbr = k1(w_ih[rows_r, 0])
    wbn = k1(w_ih[rows_n, 0])
    wu = k1(np.array([-1.0], np.float32))

    bc = dup(-(b_ih[rows_z] + b_hh[rows_z])[:, None])
    br = dup((b_ih[rows_r] + b_hh[rows_r])[:, None])
    bhn = dup(b_hh[rows_n, None])
    bin_ = dup(b_ih[rows_n, None])

    u64 = np.asarray(u, np.float64)
    with np.errstate(divide="ignore"):
        L = (np.log(u64) - np.log1p(-u64) - float(head_b[0])).astype(np.float32)
    uhats = []
    n_sites = u.shape[1]
    for c in range(N_CORES):
        Lc = L[c * B_LOCAL : (c + 1) * B_LOCAL].T  # [S, 2048]
        # [stream, site, half, NH]
        arr = Lc.reshape(n_sites, NSTREAM, 2, NH).transpose(1, 0, 2, 3)
        uhats.append(np.ascontiguousarray(arr))
    consts = dict(wc=wc, wr=wr, wn=wn, whd=whd, wbc=wbc, wbr=wbr, wbn=wbn,
                  wu=wu, bc=bc, br=br, bhn=bhn, bin=bin_)
    return consts, uhats


def make_in_maps(np_inputs):
    consts, uhats = _pack_inputs(**np_inputs)
    return [dict(consts, uhat=uhats[c]) for c in range(N_CORES)]


def kernel(u, w_ih, w_hh, b_ih, b_hh, head_w, head_b):
    from concourse.bass_utils import run_bass_kernel_spmd

    u = np.asarray(u)
    n_sites = u.shape[1]
    if n_sites not in _BUILD_CACHE:
        _BUILD_CACHE[n_sites] = _build(n_sites)
    nc = _BUILD_CACHE[n_sites]

    in_maps = make_in_maps(
        dict(u=u, w_ih=w_ih, w_hh=w_hh, b_ih=b_ih, b_hh=b_hh,
             head_w=head_w, head_b=head_b)
    )
    res = run_bass_kernel_spmd(nc, in_maps, list(range(N_CORES)))
    out = np.empty((BATCH, n_sites), np.int32)
    for c in range(N_CORES):
        bits = res.results[c]["bits"]  # [n_sites, 2048] cols s*1024+half*512+i
        out[c * B_LOCAL : (c + 1) * B_LOCAL] = bits.T.astype(np.int32)
    return out


# revision 18
# speedup vs baseline: 1.0863x; 1.0863x over previous
"""Trainium2 Bass kernel for autoregressive GRU sampling (block-diag v4).

Problem: B=16384 samples, 1024 sequential sites, hidden=64, PyTorch GRU-cell
math with gates [r,z,n], Bernoulli sampling via pre-drawn uniforms.

Data-parallel over 8 cores (2048 samples/core). Per core, 2 software-
pipelined streams of 1024 samples; within a stream the two 512-sample halves
A/B are STACKED on partitions: every elementwise tensor is [128, nh]
(rows 0-63 = half A, rows 64-127 = half B). DVE/ACT/GPSIMD are free-dim
streaming engines, so stacking halves their per-op time twice over vs the
wide [64, 2048] layout.

Matmuls use BLOCK-DIAGONAL weights so one instruction serves both halves
(PE cost is N-streaming-bound; K is free):
  P_c  = blockdiag(-Wz^T, -Wz^T) . H      K=128, M=128   (c = sigma(-a_z))
  P_c += [wbc; 0 | 0; wbc] . BITS         K=2 accumulate
  P_r, P_n, P_gx analogous
  ph[0:2] = [wh|0 ; 0|wh] . H' - I2 . UH  K=128 M=2, K=2 accumulate
Elementwise per site:
  C = sigmoid(P_c + bc), R = sigmoid(P_r + br)   [ACT, bias port]
  RG = (P_n + bhn) * R                           [DVE stt]
  NP = RG + P_gx                                 [DVE tt]
  NT = tanh(NP + bin)                            [ACT, bias port]
  E = NT - H ; U = C * E                         [GPSIMD]
  H' = H + U                                     [DVE]
  BITS' = (ph > 0)                               [DVE tensor_scalar, 1 op]
Host precomputes uhat = logit(u) - head_b (float64, clamped to +-1e4 so the
zero blocks in block-diag weights never multiply an infinity).

The two streams are emitted offset by 3 of 7 phases (software pipelining) so
each engine's in-order queue always holds ready work.
"""

import numpy as np
from contextlib import ExitStack

HIDDEN = 64
N_SITES = 1024
BATCH = 16384
N_CORES = 8
B_LOCAL = BATCH // N_CORES  # 2048
NSTREAM = 2
NH = 512  # samples per stacked half

_BUILD_CACHE = {}


def _build(n_sites: int, compile: bool = True, nstream: int = NSTREAM, nh: int = NH,
           use_gps: bool = True, nodeps: bool = False):
    import concourse.bass as bass
    import concourse.bacc as bacc
    import concourse.tile as tile
    from concourse import mybir

    f32 = mybir.dt.float32
    AF = mybir.ActivationFunctionType
    OP = mybir.AluOpType

    nc = bacc.Bacc()
    uhat_d = nc.dram_tensor("uhat", [nstream, n_sites, 2, nh], f32, kind="ExternalInput")
    wc_d = nc.dram_tensor("wc", [128, 128], f32, kind="ExternalInput")
    wr_d = nc.dram_tensor("wr", [128, 128], f32, kind="ExternalInput")
    wn_d = nc.dram_tensor("wn", [128, 128], f32, kind="ExternalInput")
    whd_d = nc.dram_tensor("whd", [128, 2], f32, kind="ExternalInput")
    wbc_d = nc.dram_tensor("wbc", [2, 128], f32, kind="ExternalInput")
    wbr_d = nc.dram_tensor("wbr", [2, 128], f32, kind="ExternalInput")
    wbn_d = nc.dram_tensor("wbn", [2, 128], f32, kind="ExternalInput")
    wu_d = nc.dram_tensor("wu", [2, 2], f32, kind="ExternalInput")
    bc_d = nc.dram_tensor("bc", [128, 1], f32, kind="ExternalInput")
    br_d = nc.dram_tensor("br", [128, 1], f32, kind="ExternalInput")
    bhn_d = nc.dram_tensor("bhn", [128, 1], f32, kind="ExternalInput")
    bin_d = nc.dram_tensor("bin", [128, 1], f32, kind="ExternalInput")
    # bits out: col = stream*2*nh + half*nh + i  (== natural sample order)
    bits_d = nc.dram_tensor("bits", [n_sites, B_LOCAL], f32, kind="ExternalOutput")

    with ExitStack() as ctx:
        tc = ctx.enter_context(tile.TileContext(nc))
        const = ctx.enter_context(tc.tile_pool(name="const", bufs=1))
        work = ctx.enter_context(tc.tile_pool(name="work", bufs=2))
        # at most 2 psum pool-sets (8 banks); streams share pairwise beyond 2
        npools = min(nstream, 2)
        psum_pools = [
            ctx.enter_context(tc.tile_pool(name=f"ps{i}", bufs=1, space="PSUM"))
            for i in range(npools)
        ]
        psum = [psum_pools[s % npools] for s in range(nstream)]

        # Weights/biases bounce through DVE copies so consumers' sem waits
        # collapse onto the DVE counter (keeps matmul <=2 sync waits).
        w_tiles = {}
        for nm, dram, shp in (
            ("wc", wc_d, [128, 128]), ("wr", wr_d, [128, 128]),
            ("wn", wn_d, [128, 128]), ("whd", whd_d, [128, 2]),
            ("wbc", wbc_d, [2, 128]), ("wbr", wbr_d, [2, 128]),
            ("wbn", wbn_d, [2, 128]), ("wu", wu_d, [2, 2]),
            ("bc", bc_d, [128, 1]), ("br", br_d, [128, 1]),
            ("bhn", bhn_d, [128, 1]), ("bin", bin_d, [128, 1]),
        ):
            raw = const.tile(shp, f32, name=f"{nm}_raw")
            nc.sync.dma_start(raw[:], dram[:])
            t = const.tile(shp, f32, name=nm)
            nc.vector.tensor_copy(t[:], raw[:])
            w_tiles[nm] = t
        wc, wr, wn, whd = (w_tiles[k] for k in ("wc", "wr", "wn", "whd"))
        wbc, wbr, wbn, wu = (w_tiles[k] for k in ("wbc", "wbr", "wbn", "wu"))
        bc, br, bhn, bin_ = (w_tiles[k] for k in ("bc", "br", "bhn", "bin"))

        H = [[const.tile([128, nh], f32, name=f"H{s}_{p}") for p in range(2)]
             for s in range(nstream)]
        BIT = [[const.tile([2, nh], f32, name=f"BIT{s}_{p}") for p in range(2)]
               for s in range(nstream)]
        UH = [[const.tile([2, nh], f32, name=f"UH{s}_{p}") for p in range(2)]
              for s in range(nstream)]
        for s in range(nstream):
            for p in range(2):
                nc.vector.memzero(H[s][p][:])
                nc.vector.memzero(BIT[s][p][:])
            for p in range(min(2, n_sites)):
                nc.sync.dma_start(UH[s][p][:], uhat_d[s, p, :, :])

        MM = nc.tensor.matmul
        eng = nc.gpsimd if use_gps else nc.vector
        SV = [{} for _ in range(nstream)]

        def emit_phase(s, t, p):
            v = SV[s]
            cur, nxt = H[s][t % 2], H[s][(t + 1) % 2]
            bcur, bnxt = BIT[s][t % 2], BIT[s][(t + 1) % 2]
            ucur = UH[s][t % 2]
            if nodeps:
                # timing experiment: break the recurrence (constant state in,
                # rotating scratch out) to expose pure engine throughput
                cur, bcur, ucur = H[s][0], BIT[s][0], UH[s][0]
                if p == 5:
                    nxt = work.tile([128, nh], f32, tag=f"HX{s}", name=f"HX{s}")
                if p == 6:
                    bnxt = work.tile([2, nh], f32, tag=f"BX{s}", name=f"BX{s}")
            if p == 0:
                P_c = psum[s].tile([128, nh], f32, tag="pc", name=f"pc{s}")
                P_r = psum[s].tile([128, nh], f32, tag="pr", name=f"pr{s}")
                P_n = psum[s].tile([128, nh], f32, tag="pn", name=f"pn{s}")
                P_gx = psum[s].tile([128, nh], f32, tag="gx", name=f"gx{s}")
                v.update(pc=P_c, pr=P_r, pn=P_n, gx=P_gx)
                MM(P_c[:], wc[:], cur[:], start=True, stop=False)
                MM(P_r[:], wr[:], cur[:], start=True, stop=False)
                MM(P_n[:], wn[:], cur[:], start=True, stop=True)
                MM(P_c[:], wbc[:], bcur[:], start=False, stop=True)
                MM(P_r[:], wbr[:], bcur[:], start=False, stop=True)
                MM(P_gx[:], wbn[:], bcur[:], start=True, stop=True)
            elif p == 1:
                Ct = work.tile([128, nh], f32, tag=f"C{s}", name=f"C{s}")
                Rt = work.tile([128, nh], f32, tag=f"R{s}", name=f"R{s}")
                nc.scalar.activation(Ct[:], v["pc"][:], AF.Sigmoid, bias=bc[:])
                nc.scalar.activation(Rt[:], v["pr"][:], AF.Sigmoid, bias=br[:])
                v.update(C=Ct, R=Rt)
            elif p == 2:
                RGt = work.tile([128, nh], f32, tag=f"RG{s}", name=f"RG{s}")
                nc.vector.scalar_tensor_tensor(
                    RGt[:], v["pn"][:], bhn[:], v["R"][:], OP.add, OP.mult
                )
                NPt = work.tile([128, nh], f32, tag=f"NP{s}", name=f"NP{s}")
                nc.vector.tensor_tensor(NPt[:], RGt[:], v["gx"][:], OP.add)
                v.update(NP=NPt)
            elif p == 3:
                NTt = work.tile([128, nh], f32, tag=f"NT{s}", name=f"NT{s}")
                nc.scalar.activation(NTt[:], v["NP"][:], AF.Tanh, bias=bin_[:])
                v.update(NT=NTt)
            elif p == 4:
                Et = work.tile([128, nh], f32, tag=f"E{s}", name=f"E{s}")
                eng.tensor_tensor(Et[:], v["NT"][:], cur[:], OP.subtract)
                Ut = work.tile([128, nh], f32, tag=f"U{s}", name=f"U{s}")
                eng.tensor_tensor(Ut[:], v["C"][:], Et[:], OP.mult)
                v.update(U=Ut)
            elif p == 5:
                nc.vector.tensor_tensor(nxt[:], cur[:], v["U"][:], OP.add)
                P_h = psum[s].tile([2, nh], f32, tag="gx", name=f"ph{s}")
                v.update(ph=P_h)
                MM(P_h[:], whd[:], nxt[:], start=True, stop=False)
                MM(P_h[:], wu[:], ucur[:], start=False, stop=True)
            elif p == 6:
                nc.vector.tensor_scalar(
                    bnxt[:], v["ph"][:], 0.0, None, OP.is_gt
                )
                base = s * 2 * nh
                nc.sync.dma_start(
                    bits_d[t : t + 1, base : base + 2 * nh], bnxt[:]
                )
                if t + 2 < n_sites and not nodeps:
                    nc.sync.dma_start(UH[s][t % 2][:], uhat_d[s, t + 2, :, :])

        # software pipeline: stream s runs OFF phases behind stream s-1 so
        # every engine's in-order queue always holds ready work.
        NPH = 7
        OFF = 3
        total = n_sites * NPH
        for G in range(total + OFF * (nstream - 1)):
            for s in range(nstream):
                g = G - OFF * s
                if 0 <= g < total:
                    emit_phase(s, g // NPH, g % NPH)

    if compile:
        nc.compile()
    return nc


def _pack_inputs(u, w_ih, w_hh, b_ih, b_hh, head_w, head_b,
                 nstream=NSTREAM, nh=NH):
    H = HIDDEN
    w_ih = np.asarray(w_ih, np.float32)
    w_hh = np.asarray(w_hh, np.float32)
    b_ih = np.asarray(b_ih, np.float32)
    b_hh = np.asarray(b_hh, np.float32)
    head_w = np.asarray(head_w, np.float32)
    head_b = np.asarray(head_b, np.float32)

    rows_r = slice(0, H)
    rows_z = slice(H, 2 * H)
    rows_n = slice(2 * H, 3 * H)

    def bd(x):  # [64, 64] -> [128, 128] block-diagonal
        out = np.zeros((128, 128), np.float32)
        out[0:64, 0:64] = x
        out[64:128, 64:128] = x
        return out

    wc = bd(-w_hh[rows_z, :].T)
    wr = bd(w_hh[rows_r, :].T)
    wn = bd(w_hh[rows_n, :].T)
    whd = np.zeros((128, 2), np.float32)
    whd[0:64, 0] = head_w[0, :]
    whd[64:128, 1] = head_w[0, :]

    def k2(vec_m):  # [64] -> [2, 128]: row 0 -> cols 0:64, row 1 -> cols 64:128
        out = np.zeros((2, 128), np.float32)
        out[0, 0:64] = vec_m
        out[1, 64:128] = vec_m
        return out

    wbc = k2(-w_ih[rows_z, 0])
    wbr = k2(w_ih[rows_r, 0])
    wbn = k2(w_ih[rows_n, 0])
    wu = (-np.eye(2)).astype(np.float32)

    def dup(x):
        return np.concatenate([x, x], axis=0).astype(np.float32)

    bc = dup(-(b_ih[rows_z] + b_hh[rows_z])[:, None])
    br = dup((b_ih[rows_r] + b_hh[rows_r])[:, None])
    bhn = dup(b_hh[rows_n, None])
    bin_ = dup(b_ih[rows_n, None])

    u64 = np.asarray(u, np.float64)
    with np.errstate(divide="ignore"):
        L = np.log(u64) - np.log1p(-u64) - float(head_b[0])
    # clamp: the block-diag zero weights would turn inf into NaN in psum
    L = np.clip(L, -1e4, 1e4).astype(np.float32)
    uhats = []
    n_sites = u.shape[1]
    for c in range(N_CORES):
        Lc = L[c * B_LOCAL : (c + 1) * B_LOCAL].T  # [S, 2048]
        arr = Lc.reshape(n_sites, nstream, 2, nh).transpose(1, 0, 2, 3)
        uhats.append(np.ascontiguousarray(arr))
    consts = dict(wc=wc, wr=wr, wn=wn, whd=whd, wbc=wbc, wbr=wbr, wbn=wbn,
                  wu=wu, bc=bc, br=br, bhn=bhn, bin=bin_)
    return consts, uhats


def make_in_maps(np_inputs, nstream=NSTREAM, nh=NH):
    consts, uhats = _pack_inputs(**np_inputs, nstream=nstream, nh=nh)
    return [dict(consts, uhat=uhats[c]) for c in range(N_CORES)]


def kernel(u, w_ih, w_hh, b_ih, b_hh, head_w, head_b):
    from concourse.bass_utils import run_bass_kernel_spmd

    u = np.asarray(u)
    n_sites = u.shape[1]
    if n_sites not in _BUILD_CACHE:
        _BUILD_CACHE[n_sites] = _build(n_sites)
    nc = _BUILD_CACHE[n_sites]

    in_maps = make_in_maps(
        dict(u=u, w_ih=w_ih, w_hh=w_hh, b_ih=b_ih, b_hh=b_hh,
             head_w=head_w, head_b=head_b)
    )
    res = run_bass_kernel_spmd(nc, in_maps, list(range(N_CORES)))
    out = np.empty((BATCH, n_sites), np.int32)
    for c in range(N_CORES):
        out[c * B_LOCAL : (c + 1) * B_LOCAL] = res.results[c]["bits"].T.astype(np.int32)
    return out
